# revision 9
# baseline (speedup 1.0000x reference)
"""Causal single-head attention (B=4, S=4096, d=1024) on 8 Trainium2 NeuronCores.

Sharding: 8 cores = 4 batches x 2 sequence-groups.  Per batch, the causal
q-blocks are paired so each core gets a uniform static program with slot
coverages COV; causal masking and padding are handled by a data-driven mask
    A = exp(s/sqrt(d)) * (I - J <= delta)
so all 8 cores run a single SPMD program; only the input data differs per core.

v2 (this file): everything the PE touches is bf16.
  - Host converts x^T and Wq/Wk/Wv to bf16 (PE bf16 matmul = same 1 col/cycle
    as fp32r, but half the DMA/SBUF bytes; rel-err budget 2e-2 >> bf16's ~5e-3).
  - kT and qT live in SBUF for the whole kernel ([128, DC, S] + [128, DC, QROWS]
    bf16 = 96 KiB/partition): k/q projection's PSUM->SBUF ACT copies write them
    directly, so the attention phase has NO kt/qt DMA at all.
  - v goes to DRAM in bf16 and is streamed during attention (~20 MB/core vs
    ~88 MB/core f32 before) -- far below the ~360 GB/s/core HBM ceiling the
    old version was pinned at.

Math (per core):
  kT = Wk^T x^T ([d, s], d on partitions) -> SBUF bf16; v = x Wv -> DRAM bf16;
  qT = Wq^T xq^T -> SBUF bf16.
  Per slot (QBLK q rows), per k-tile (128 rows):
    sT[k, q]   = sum_e kT[e,k] qT[e,q]          (PE, bf16, fp32 PSUM)
    A[k, q]    = exp(sT/32) * mask              (ACT exp PSUM->SBUF bf16, DVE)
    out[q, d] += A[:,qm]^T v[k, d]              (PE, accumulated in PSUM)
    den[q]    += A[:,qm]^T ones                 (PE, N=2)
  out /= den  (DVE reciprocal + ACT Copy with per-partition scale)
No running max is needed: scores are ~N(0,1) after the 1/32 scale, and exp
without max-subtraction is safe (max score ~5.5 -> exp ~250, well in range).

The attention inner loop is software-pipelined: every DMA is issued at least
one step ahead, and visit i's attn@v/denominator matmuls are emitted after
visit i+1's score matmuls so the PE never waits on the ACT exp.

Toolchain notes: tiny PE "pre-touch" matmuls observe each fresh DMA tick so
real matmuls keep a single sync wait; bacc.Bacc legalizes any remaining
multi-wait instructions via event semaphores.
"""

import contextlib
import math

import numpy as np
from ml_dtypes import bfloat16

import concourse.bass as bass  # noqa: F401
import concourse.mybir as mybir
import concourse.tile as tile
from concourse import bacc
from concourse.bass_utils import run_bass_kernel_spmd

F32 = mybir.dt.float32
BF16 = mybir.dt.bfloat16
AF = mybir.ActivationFunctionType
ALU = mybir.AluOpType

CFG_FULL = dict(S=4096, D=1024, QBLK=512, COV=(8, 16, 24, 32))
Q0_FULL = {0: (0, 1536, 2048, 3584), 1: (512, 1024, 2560, 3072)}
RG_FULL = [[0, 1], [2, 3], [4, 5], [6, 7]]
B_FULL = 4
USE_RG = False  # collectives too slow for kv-dedup (2-rank AG ~34 GB/s)


def build_nc(S, D, QBLK, COV, reps=1, rg=None):
    """Build the single-core Bass program (identical across all cores)."""
    assert rg is None
    DC = D // 128
    M = QBLK // 128
    nslots = len(COV)
    QROWS = nslots * QBLK
    DHALF = min(512, D)
    NH = D // DHALF
    SBLK = min(512, S)
    NSB = S // SBLK
    maxcov = max(COV)
    assert maxcov == S // 128
    scale = 1.0 / math.sqrt(D)

    nc = bacc.Bacc("TRN2", target_bir_lowering=False)
    xT_d = nc.dram_tensor("xT", [D, S], BF16, kind="ExternalInput")
    xTq_d = nc.dram_tensor("xTq", [D, QROWS], BF16, kind="ExternalInput")
    wq_d = nc.dram_tensor("Wq", [D, D], BF16, kind="ExternalInput")
    wk_d = nc.dram_tensor("Wk", [D, D], BF16, kind="ExternalInput")
    wv_d = nc.dram_tensor("Wv", [D, D], BF16, kind="ExternalInput")
    ij_d = nc.dram_tensor("IJ", [128, QBLK], F32, kind="ExternalInput")
    dl_d = nc.dram_tensor("delta", [128, nslots * maxcov], F32,
                          kind="ExternalInput")
    ones_d = nc.dram_tensor("ones", [128, 2], BF16, kind="ExternalInput")
    out_d = nc.dram_tensor("out", [QROWS, D], F32, kind="ExternalOutput")

    def dpart(ap):
        return ap.rearrange("(c p) n -> p c n", p=128)

    with tile.TileContext(nc) as tc:
        with tc.tile_pool(name="dram", bufs=1, space="DRAM") as dram, \
             tc.tile_pool(name="dummy", bufs=1, space="PSUM") as dummypool:
            v_i = dram.tile([S, D], BF16, name="v_i")
            dummy_ps = dummypool.tile([128, 2], F32, name="dummy_ps",
                                      tag="dummy")

            def touch(cols2):
                # Tiny matmul reading two columns of a freshly written SBUF
                # tile: absorbs the DMA-completion wait so the real matmuls
                # keep a single sync wait each.
                nc.tensor.matmul(dummy_ps[0:1, 0:2], cols2[:, 0:1], cols2,
                                 start=True, stop=True)

            _loop = (tc.For_i(0, reps, 1) if reps > 1
                     else contextlib.nullcontext())
            with _loop, tc.tile_pool(name="persist", bufs=1) as pers:
                kT_sb = pers.tile([128, DC, S], BF16, name="kT", tag="kT")
                qT_sb = pers.tile([128, DC, QROWS], BF16, name="qT",
                                  tag="qT")
                # ---------------- Phase 1: projections ----------------
                with (
                    tc.tile_pool(name="w", bufs=1) as wpool,
                    tc.tile_pool(name="xt", bufs=3) as xtpool,
                    tc.tile_pool(name="vst", bufs=4) as vspool,
                    tc.tile_pool(name="ppsum", bufs=7, space="PSUM") as ppsum,
                ):
                    w_sb = {}

                    def w_load(name, wd):
                        w_sb[name] = wpool.tile([128, DC, D], BF16,
                                                name=f"w{name}",
                                                tag=f"w{name}")
                        nc.sync.dma_start(out=w_sb[name], in_=dpart(wd[:, :]))
                        touch(w_sb[name][:, 0, 0:2])

                    def pcopy(dst, src):
                        # PSUM->SBUF on ACT (casts f32 PSUM to bf16 dst).
                        nc.scalar.copy(out=dst, in_=src)

                    # one prefetched stream of xT blocks.  Interleave kv
                    # and q jobs so attention slot s's inputs are stored as
                    # early as possible.
                    kvjobs = [("kv", sb) for sb in range(NSB)]
                    qjobs = [("q", s) for s in range(nslots)]
                    jobs = []
                    per = max(1, NSB // nslots)
                    for s in range(nslots):
                        jobs += kvjobs[s * per:(s + 1) * per]
                        jobs.append(qjobs[s])
                    jobs += kvjobs[nslots * per:]

                    def xt_load(job):
                        kind, idx = job
                        blk = SBLK if kind == "kv" else QBLK
                        src = xT_d if kind == "kv" else xTq_d
                        xt = xtpool.tile([128, DC, blk], BF16, name="xt",
                                         tag="xt")
                        nc.sync.dma_start(
                            out=xt,
                            in_=dpart(src[:, idx * blk:(idx + 1) * blk]))
                        return xt

                    # first x block before the weights: the first matmul
                    # needs xt0 AND Wk, so don't serialize xt0 behind all
                    # three W loads on the SP DMA queue.
                    xts = {0: xt_load(jobs[0])}
                    w_load("k", wk_d)
                    w_load("v", wv_d)
                    w_load("q", wq_d)
                    for jidx, job in enumerate(jobs):
                        if jidx + 1 < len(jobs):
                            xts[jidx + 1] = xt_load(jobs[jidx + 1])
                        xt = xts.pop(jidx)
                        touch(xt[:, 0, 0:2])
                        kind, idx = job
                        if kind == "kv":
                            for co in range(DC):
                                ps = ppsum.tile([128, SBLK], F32, name="pp",
                                                tag="pp")
                                for ci in range(DC):
                                    nc.tensor.matmul(
                                        ps,
                                        w_sb["k"][:, ci,
                                                  co * 128:(co + 1) * 128],
                                        xt[:, ci, :],
                                        start=(ci == 0), stop=(ci == DC - 1))
                                pcopy(kT_sb[:, co,
                                            idx * SBLK:(idx + 1) * SBLK], ps)
                            for m in range(SBLK // 128):
                                vs = vspool.tile([128, D], BF16, name="vs",
                                                 tag="vs")
                                for h in range(NH):
                                    ps = ppsum.tile([128, DHALF], F32,
                                                    name="pp", tag="pp")
                                    for ci in range(DC):
                                        nc.tensor.matmul(
                                            ps,
                                            xt[:, ci, m * 128:(m + 1) * 128],
                                            w_sb["v"][:, ci, h * DHALF:
                                                      (h + 1) * DHALF],
                                            start=(ci == 0),
                                            stop=(ci == DC - 1))
                                    pcopy(vs[:, h * DHALF:(h + 1) * DHALF],
                                          ps)
                                nc.scalar.dma_start(
                                    out=v_i[idx * SBLK + m * 128:
                                            idx * SBLK + (m + 1) * 128, :],
                                    in_=vs)
                        else:
                            for co in range(DC):
                                ps = ppsum.tile([128, QBLK], F32, name="pp",
                                                tag="pp")
                                for ci in range(DC):
                                    nc.tensor.matmul(
                                        ps,
                                        w_sb["q"][:, ci,
                                                  co * 128:(co + 1) * 128],
                                        xt[:, ci, :],
                                        start=(ci == 0), stop=(ci == DC - 1))
                                pcopy(qT_sb[:, co,
                                            idx * QBLK:(idx + 1) * QBLK], ps)

                # ---------------- Phase 2: attention ----------------
                with (
                    tc.tile_pool(name="at", bufs=maxcov + 16) as apool,
                    tc.tile_pool(name="vt", bufs=4) as vtpool,
                    tc.tile_pool(name="ot", bufs=4) as otpool,
                    tc.tile_pool(name="cm", bufs=2) as cmpool,
                    tc.tile_pool(name="sm", bufs=1) as smpool,
                    tc.tile_pool(name="rc", bufs=2) as rcpool,
                    tc.tile_pool(name="spsum", bufs=2, space="PSUM") as spsum,
                    tc.tile_pool(name="opsum", bufs=M, space="PSUM") as opsum,
                    tc.tile_pool(name="dpsum", bufs=1, space="PSUM") as dpsum,
                ):
                    ij_sb = smpool.tile([128, QBLK], F32, name="ij", tag="ij")
                    nc.sync.dma_start(out=ij_sb, in_=ij_d[:, :])
                    dl_sb = smpool.tile([128, nslots * maxcov], F32,
                                        name="dl", tag="dl")
                    nc.sync.dma_start(out=dl_sb, in_=dl_d[:, :])
                    ones_sb = smpool.tile([128, 2], BF16, name="ones",
                                          tag="ones")
                    nc.sync.dma_start(out=ones_sb, in_=ones_d[:, :])
                    touch(ones_sb)

                    def vt_load(i, h):
                        vt = vtpool.tile([128, 2, DHALF], BF16, name="vt",
                                         tag="vt")
                        nc.sync.dma_start(
                            out=vt,
                            in_=v_i[i * 128:i * 128 + 256,
                                    h * DHALF:(h + 1) * DHALF]
                            .rearrange("(t p) d -> p t d", p=128))
                        return vt

                    for s in range(nslots):
                        cov = COV[s]
                        qt = qT_sb[:, :, s * QBLK:(s + 1) * QBLK]
                        po = [opsum.tile([128, DHALF], F32, name="po",
                                         tag="po") for _ in range(M)]
                        pd = dpsum.tile([128, 2 * M], F32, name="pd",
                                        tag="pd")

                        def consume(i, at, vts, hh, dst, fresh_touch):
                            # attn@v + denominator matmuls for visit i
                            vt = vts[i // 2]
                            if fresh_touch:
                                touch(vt[:, 0, 0:2])
                            for m in range(M):
                                nc.tensor.matmul(
                                    dst[m],
                                    at[:, m * 128:(m + 1) * 128],
                                    vt[:, i % 2, :],
                                    start=(i == 0), stop=(i == cov - 1))
                            if hh == 0:
                                for m in range(M):
                                    nc.tensor.matmul(
                                        pd[:, 2 * m:2 * m + 2],
                                        at[:, m * 128:(m + 1) * 128],
                                        ones_sb[:, :],
                                        start=(i == 0 and m == 0),
                                        stop=(i == cov - 1 and m == M - 1))

                        # ---- sweep 1: scores + exp + attn@v(d-half 0) ----
                        a_tiles = []
                        vts = {0: vt_load(0, 0)}
                        prev = None
                        for i in range(cov):
                            if i % 2 == 0 and i + 2 < cov:
                                vts[(i + 2) // 2] = vt_load(i + 2, 0)
                            ps = spsum.tile([128, QBLK], F32, name="ps",
                                            tag="ps")
                            for ci in range(DC):
                                nc.tensor.matmul(
                                    ps,
                                    kT_sb[:, ci, i * 128:(i + 1) * 128],
                                    qt[:, ci, :],
                                    start=(ci == 0), stop=(ci == DC - 1))
                            at = apool.tile([128, QBLK], BF16, name="at",
                                            tag="at")
                            nc.scalar.activation(
                                out=at, in_=ps, func=AF.Exp, scale=scale)
                            if i >= cov - 2 * M:
                                cm = cmpool.tile([128, QBLK], BF16,
                                                 name="cm", tag="cm")
                                nc.vector.tensor_scalar(
                                    out=cm, in0=ij_sb,
                                    scalar1=dl_sb[:, s * maxcov + i:
                                                  s * maxcov + i + 1],
                                    scalar2=None, op0=ALU.is_le)
                                nc.vector.tensor_mul(out=at, in0=at, in1=cm)
                            a_tiles.append(at)
                            if prev is not None:
                                consume(prev, a_tiles[prev], vts, 0, po,
                                        prev % 2 == 0)
                            prev = i
                        consume(prev, a_tiles[prev], vts, 0, po,
                                prev % 2 == 0)

                        rc = rcpool.tile([128, 2 * M], F32, name="rc",
                                         tag="rc")
                        nc.vector.reciprocal(out=rc, in_=pd)

                        def norm_store(dst_po, h):
                            # on DVE (+ DVE's DMA queue) so the ACT queue
                            # stays exp-only: the next slot's first exp must
                            # not sit behind norm/store work, or the PE
                            # stalls on the score-PSUM WAR.
                            for m in range(M):
                                ot = otpool.tile([128, DHALF], F32,
                                                 name="ot", tag="ot")
                                nc.vector.tensor_scalar(
                                    out=ot, in0=dst_po[m],
                                    scalar1=rc[:, 2 * m:2 * m + 1],
                                    scalar2=None, op0=ALU.mult)
                                nc.gpsimd.dma_start(
                                    out=out_d[s * QBLK + m * 128:
                                              s * QBLK + (m + 1) * 128,
                                              h * DHALF:(h + 1) * DHALF],
                                    in_=ot)

                        norm_store(po, 0)

                        # ---- sweep 2: attn@v for remaining d-halves ----
                        for h in range(1, NH):
                            po2 = [opsum.tile([128, DHALF], F32, name="po",
                                              tag="po") for _ in range(M)]
                            vts = {0: vt_load(0, h)}
                            if cov > 2:
                                vts[1] = vt_load(2, h)
                            for i in range(cov):
                                if i % 2 == 0 and i + 4 < cov:
                                    vts[(i + 4) // 2] = vt_load(i + 4, h)
                                consume(i, a_tiles[i], vts, h, po2,
                                        i % 2 == 0)
                            norm_store(po2, h)
    # Bacc legalization: splits >1 sync waits into event semaphores.
    nc.compile()
    return nc


def host_core_inputs(x_b, Wq, Wk, Wv, q0s, S, D, QBLK, COV, half=None):
    """Input map for one core (half is accepted for test.py compat; unused)."""
    nslots = len(COV)
    maxcov = max(COV)
    xT = np.ascontiguousarray(x_b.T.astype(bfloat16))
    xTq = np.ascontiguousarray(
        np.concatenate([x_b[q0:q0 + QBLK] for q0 in q0s], axis=0).T
        .astype(bfloat16))
    ij = (np.arange(128, dtype=np.float32)[:, None]
          - np.arange(QBLK, dtype=np.float32)[None, :])
    ij = np.ascontiguousarray(np.broadcast_to(ij, (128, QBLK)))
    delta = np.empty((nslots, maxcov), dtype=np.float32)
    for s, q0 in enumerate(q0s):
        delta[s, :] = q0 - 128.0 * np.arange(maxcov, dtype=np.float32)
    delta = np.ascontiguousarray(
        np.broadcast_to(delta.reshape(1, -1), (128, nslots * maxcov)))
    return {
        "xT": xT, "xTq": xTq,
        "Wq": np.ascontiguousarray(Wq.astype(bfloat16)),
        "Wk": np.ascontiguousarray(Wk.astype(bfloat16)),
        "Wv": np.ascontiguousarray(Wv.astype(bfloat16)),
        "IJ": ij, "delta": delta,
        "ones": np.ones((128, 2), bfloat16),
    }


_NC_CACHE = {}


def _get_nc(key, cfg, **kw):
    if key not in _NC_CACHE:
        _NC_CACHE[key] = build_nc(**cfg, **kw)
    return _NC_CACHE[key]


def run_full(x, Wq, Wk, Wv, trace=False, trace_cores=None):
    """Run the 8-core kernel on full inputs; returns (out, BassKernelResults)."""
    cfg = CFG_FULL
    S, D, QBLK, COV = cfg["S"], cfg["D"], cfg["QBLK"], cfg["COV"]
    x = np.asarray(x, np.float32)
    Wq = np.asarray(Wq, np.float32)
    Wk = np.asarray(Wk, np.float32)
    Wv = np.asarray(Wv, np.float32)
    B = x.shape[0]
    assert (B, x.shape[1], x.shape[2]) == (B_FULL, S, D)

    nc = _get_nc("full", cfg)
    in_maps = []
    for b in range(B):
        for h in range(2):
            in_maps.append(host_core_inputs(
                x[b], Wq, Wk, Wv, Q0_FULL[h], S, D, QBLK, COV))
    res = run_bass_kernel_spmd(
        nc, in_maps, list(range(2 * B)), trace=trace,
        trace_cores=trace_cores)
    out = np.empty((B, S, D), np.float32)
    for b in range(B):
        for h in range(2):
            o = np.asarray(res.results[2 * b + h]["out"])
            for s, q0 in enumerate(Q0_FULL[h]):
                out[b, q0:q0 + QBLK] = o[s * QBLK:(s + 1) * QBLK]
    return out, res


def kernel(x, Wq, Wk, Wv):
    out, _ = run_full(x, Wq, Wk, Wv)
    return out


# revision 12
# speedup vs baseline: 1.2060x; 1.2060x over previous
"""Causal single-head attention (B=4, S=4096, d=1024) on 8 Trainium2 NeuronCores.

Sharding: 8 cores = 4 batches x 2 groups.  Per batch, the causal q-blocks are
paired so each core gets a uniform static program with slot coverages COV;
causal masking and padding use a data-driven mask
    A = exp(s/sqrt(d)) * (I - J <= delta)
so all 8 cores run one SPMD program; only input data differs per core.

v3: bf16 everywhere + SBUF-resident kT/qT + kv-projection dedup.
  - Host converts x^T and Wq/Wk/Wv to bf16 (PE bf16 matmul streams 1 col/cycle
    like fp32r, but the serialized per-matmul weight load is 2x faster via FWL
    and all DMA/SBUF bytes halve; rel-err ~5e-3 << the 2e-2 budget).
  - kv dedup: the two cores of a batch each project HALF the batch's k/v rows
    (core h takes rows [1024c + 512h, 1024c + 512(h+1)) of each 1024-row chunk
    c) and exchange per-chunk via pairwise AllGather (bf16, 1 MB in / 2 MB out
    per tensor) on the gpsimd queue, overlapped with the next chunk's
    projection matmuls.  This halves the kv projection matmul count, the
    dominant per-matmul-issue cost.
  - kT lives in SBUF for the whole kernel ([128, DC, S] bf16, filled from the
    AllGather output), qT likewise (written directly by q-projection's
    PSUM->SBUF copies); v is streamed bf16 from the AllGather output during
    attention.  Attention-phase DMA is ~25 MB/core, far under the ~360 GB/s
    HBM ceiling the f32 version was pinned at.

Math (per core), per slot (QBLK q rows), per k-tile (128 rows):
    sT[k, q]   = sum_e kT[e,k] qT[e,q]          (PE, bf16, fp32 PSUM)
    A[k, q]    = exp(sT/32) * mask              (ACT exp PSUM->SBUF bf16, DVE)
    out[q, d] += A[:,qm]^T v[k, d]              (PE, accumulated in PSUM)
    den[q]    += A[:,qm]^T ones                 (PE, N=2)
    out /= den  (DVE reciprocal + DVE per-partition scale, store via SP queue)
No running max: scores are ~N(0,1) after the 1/32 scale (max exp ~250).

The attention inner loop is software-pipelined: every DMA is issued at least
one step ahead, and visit i's attn@v/denominator matmuls are emitted after
visit i+1's score matmuls so the PE never waits on the ACT exp.  Queue
placement keeps the ACT queue exp-only in attention (norm on DVE, output
stores on SP, collectives on gpsimd).

Collectives cannot sit inside a hardware loop, so timing builds (reps>1)
unroll the whole body; reps stays small.
"""

import math

import numpy as np
from ml_dtypes import bfloat16

import concourse.bass as bass  # noqa: F401
import concourse.mybir as mybir
import concourse.tile as tile
from concourse import bacc
from concourse.bass_utils import run_bass_kernel_spmd

F32 = mybir.dt.float32
BF16 = mybir.dt.bfloat16
AF = mybir.ActivationFunctionType
ALU = mybir.AluOpType

CFG_FULL = dict(S=4096, D=1024, QBLK=512, COV=(8, 16, 24, 32))
Q0_FULL = {0: (0, 1536, 2048, 3584), 1: (512, 1024, 2560, 3072)}
RG_FULL = [[0, 1], [2, 3], [4, 5], [6, 7]]
B_FULL = 4
USE_RG = True   # pairwise kv-projection dedup via chunked AllGather
NCHUNK = 4      # kv exchange chunks (1024 batch rows each)


def build_nc(S, D, QBLK, COV, reps=1, rg=RG_FULL):
    """Build the single-core Bass program (identical across all cores)."""
    DC = D // 128
    M = QBLK // 128
    nslots = len(COV)
    QROWS = nslots * QBLK
    DHALF = min(512, D)
    NH = D // DHALF
    maxcov = max(COV)
    assert maxcov == S // 128
    scale = 1.0 / math.sqrt(D)
    CROWS = S // NCHUNK          # batch rows per exchange chunk
    HROWS = CROWS // 2           # rows this core projects per chunk
    use_ag = rg is not None

    nc = bacc.Bacc("TRN2", target_bir_lowering=False,
                   num_devices=(8 if use_ag else None))
    # xTkv: the kv rows this core projects, chunk-major ([D, S/2] comb of the
    # batch; full [D, S] when not deduping).  xTq: this core's q rows.
    xkv_cols = S // 2 if use_ag else S
    xkv_d = nc.dram_tensor("xTkv", [D, xkv_cols], BF16, kind="ExternalInput")
    xTq_d = nc.dram_tensor("xTq", [D, QROWS], BF16, kind="ExternalInput")
    wq_d = nc.dram_tensor("Wq", [D, D], BF16, kind="ExternalInput")
    wk_d = nc.dram_tensor("Wk", [D, D], BF16, kind="ExternalInput")
    wv_d = nc.dram_tensor("Wv", [D, D], BF16, kind="ExternalInput")
    ij_d = nc.dram_tensor("IJ", [128, QBLK], F32, kind="ExternalInput")
    dl_d = nc.dram_tensor("delta", [128, nslots * maxcov], F32,
                          kind="ExternalInput")
    ones_d = nc.dram_tensor("ones", [128, 2], BF16, kind="ExternalInput")
    out_d = nc.dram_tensor("out", [QROWS, D], F32, kind="ExternalOutput")

    def dpart(ap):
        return ap.rearrange("(c p) n -> p c n", p=128)

    with tile.TileContext(nc) as tc:
        with tc.tile_pool(name="dram", bufs=1, space="DRAM") as dram, \
             tc.tile_pool(name="dummy", bufs=1, space="PSUM") as dummypool:
            if use_ag:
                kin = [dram.tile([DC, 128, HROWS], BF16, name=f"kin{c}")
                       for c in range(NCHUNK)]
                kout = [dram.tile([2, DC, 128, HROWS], BF16,
                                  name=f"kout{c}")
                        for c in range(NCHUNK)]
                vin = [dram.tile([HROWS, D], BF16, name=f"vin{c}")
                       for c in range(NCHUNK)]
                vout = [dram.tile([2, HROWS, D], BF16, name=f"vout{c}")
                        for c in range(NCHUNK)]
            else:
                v_i = dram.tile([S, D], BF16, name="v_i")
            dummy_ps = dummypool.tile([128, 2], F32, name="dummy_ps",
                                      tag="dummy")

            def touch(cols2):
                # Tiny matmul reading two columns of a freshly written SBUF
                # tile: absorbs the DMA-completion wait so the real matmuls
                # keep a single sync wait each.
                nc.tensor.matmul(dummy_ps[0:1, 0:2], cols2[:, 0:1], cols2,
                                 start=True, stop=True)

            def body(rep):
              with tc.tile_pool(name="persist", bufs=1) as pers:
                kT_sb = pers.tile([128, DC, S], BF16, name="kT", tag="kT")
                qT_sb = pers.tile([128, DC, QROWS], BF16, name="qT",
                                  tag="qT")
                # ---------------- Phase 1: projections ----------------
                with (
                    tc.tile_pool(name="w", bufs=1) as wpool,
                    tc.tile_pool(name="xt", bufs=3) as xtpool,
                    tc.tile_pool(name="kst", bufs=4) as kspool,
                    tc.tile_pool(name="vst", bufs=4) as vspool,
                    tc.tile_pool(name="ppsum", bufs=7, space="PSUM") as ppsum,
                ):
                    w_sb = {}

                    def w_load(name, wd):
                        w_sb[name] = wpool.tile([128, DC, D], BF16,
                                                name=f"w{name}",
                                                tag=f"w{name}")
                        nc.sync.dma_start(out=w_sb[name], in_=dpart(wd[:, :]))
                        touch(w_sb[name][:, 0, 0:2])

                    def pcopy(dst, src):
                        # PSUM->SBUF on ACT (casts f32 PSUM to bf16 dst).
                        nc.scalar.copy(out=dst, in_=src)

                    if use_ag:
                        kvjobs = [("kv", c) for c in range(NCHUNK)]
                        nkvb = NCHUNK
                        kvblk = HROWS
                    else:
                        kvjobs = [("kv", c) for c in range(S // 512)]
                        nkvb = S // 512
                        kvblk = 512
                    jobs = kvjobs + [("q", s) for s in range(nslots)]

                    def xt_load(job):
                        kind, idx = job
                        blk = kvblk if kind == "kv" else QBLK
                        src = xkv_d if kind == "kv" else xTq_d
                        xt = xtpool.tile([128, DC, blk], BF16, name="xt",
                                         tag="xt")
                        nc.sync.dma_start(
                            out=xt,
                            in_=dpart(src[:, idx * blk:(idx + 1) * blk]))
                        return xt

                    # first x block before the weights: the first matmul
                    # needs xt0 AND Wk, so don't serialize xt0 behind all
                    # three W loads on the SP DMA queue.
                    xts = {0: xt_load(jobs[0])}
                    w_load("k", wk_d)
                    w_load("v", wv_d)
                    w_load("q", wq_d)
                    for jidx, job in enumerate(jobs):
                        if jidx + 1 < len(jobs):
                            xts[jidx + 1] = xt_load(jobs[jidx + 1])
                        xt = xts.pop(jidx)
                        touch(xt[:, 0, 0:2])
                        kind, idx = job
                        if kind == "kv":
                            nmm = kvblk // 128
                            for co in range(DC):
                                ps = ppsum.tile([128, kvblk], F32, name="pp",
                                                tag="pp")
                                for ci in range(DC):
                                    nc.tensor.matmul(
                                        ps,
                                        w_sb["k"][:, ci,
                                                  co * 128:(co + 1) * 128],
                                        xt[:, ci, :],
                                        start=(ci == 0), stop=(ci == DC - 1))
                                if use_ag:
                                    ks = kspool.tile([128, kvblk], BF16,
                                                     name="ks", tag="ks")
                                    pcopy(ks, ps)
                                    nc.scalar.dma_start(
                                        out=kin[idx][co, :, :], in_=ks)
                                else:
                                    pcopy(kT_sb[:, co, idx * kvblk:
                                                (idx + 1) * kvblk], ps)
                            for m in range(nmm):
                                vs = vspool.tile([128, D], BF16, name="vs",
                                                 tag="vs")
                                for h in range(NH):
                                    ps = ppsum.tile([128, DHALF], F32,
                                                    name="pp", tag="pp")
                                    for ci in range(DC):
                                        nc.tensor.matmul(
                                            ps,
                                            xt[:, ci, m * 128:(m + 1) * 128],
                                            w_sb["v"][:, ci, h * DHALF:
                                                      (h + 1) * DHALF],
                                            start=(ci == 0),
                                            stop=(ci == DC - 1))
                                    pcopy(vs[:, h * DHALF:(h + 1) * DHALF],
                                          ps)
                                if use_ag:
                                    nc.scalar.dma_start(
                                        out=vin[idx][m * 128:(m + 1) * 128,
                                                     :],
                                        in_=vs)
                                else:
                                    nc.scalar.dma_start(
                                        out=v_i[idx * kvblk + m * 128:
                                                idx * kvblk + (m + 1) * 128,
                                                :],
                                        in_=vs)
                            if use_ag:
                                # everything AG-dependent stays on the Pool
                                # queue so no compute engine ever queue-waits
                                # on a collective; the PE picks up the
                                # dependency via a per-slot touch in phase 2.
                                c = idx
                                nc.gpsimd.collective_compute(
                                    "AllGather", ALU.bypass,
                                    replica_groups=rg,
                                    ins=[kin[c][:, :, :]],
                                    outs=[kout[c][:, :, :, :]])
                                for half in range(2):
                                    nc.gpsimd.dma_start(
                                        out=kT_sb[:, :,
                                                  c * CROWS + half * HROWS:
                                                  c * CROWS +
                                                  (half + 1) * HROWS],
                                        in_=kout[c][half]
                                        .rearrange("c p n -> p c n"))
                                nc.gpsimd.collective_compute(
                                    "AllGather", ALU.bypass,
                                    replica_groups=rg,
                                    ins=[vin[c][:, :]],
                                    outs=[vout[c][:, :, :]])
                        else:
                            for co in range(DC):
                                ps = ppsum.tile([128, QBLK], F32, name="pp",
                                                tag="pp")
                                for ci in range(DC):
                                    nc.tensor.matmul(
                                        ps,
                                        w_sb["q"][:, ci,
                                                  co * 128:(co + 1) * 128],
                                        xt[:, ci, :],
                                        start=(ci == 0), stop=(ci == DC - 1))
                                pcopy(qT_sb[:, co,
                                            idx * QBLK:(idx + 1) * QBLK], ps)

                # ---------------- Phase 2: attention ----------------
                with (
                    tc.tile_pool(name="at", bufs=maxcov + 16) as apool,
                    tc.tile_pool(name="vt", bufs=6) as vtpool,
                    tc.tile_pool(name="ot", bufs=4) as otpool,
                    tc.tile_pool(name="cm", bufs=2) as cmpool,
                    tc.tile_pool(name="sm", bufs=1) as smpool,
                    tc.tile_pool(name="rc", bufs=2) as rcpool,
                    tc.tile_pool(name="spsum", bufs=2, space="PSUM") as spsum,
                    tc.tile_pool(name="opsum", bufs=M, space="PSUM") as opsum,
                    tc.tile_pool(name="dpsum", bufs=1, space="PSUM") as dpsum,
                ):
                    ij_sb = smpool.tile([128, QBLK], F32, name="ij", tag="ij")
                    nc.sync.dma_start(out=ij_sb, in_=ij_d[:, :])
                    dl_sb = smpool.tile([128, nslots * maxcov], F32,
                                        name="dl", tag="dl")
                    nc.sync.dma_start(out=dl_sb, in_=dl_d[:, :])
                    ones_sb = smpool.tile([128, 2], BF16, name="ones",
                                          tag="ones")
                    nc.sync.dma_start(out=ones_sb, in_=ones_d[:, :])
                    touch(ones_sb)

                    def vt_load(i, h):
                        vt = vtpool.tile([128, 2, DHALF], BF16, name="vt",
                                         tag="vt")
                        if use_ag:
                            c, loc = divmod(i * 128, CROWS)
                            half, loc = divmod(loc, HROWS)
                            src = vout[c][half, loc:loc + 256,
                                          h * DHALF:(h + 1) * DHALF]
                        else:
                            src = v_i[i * 128:i * 128 + 256,
                                      h * DHALF:(h + 1) * DHALF]
                        nc.sync.dma_start(
                            out=vt, in_=src.rearrange("(t p) d -> p t d",
                                                      p=128))
                        return vt

                    for s in range(nslots):
                        cov = COV[s]
                        if use_ag:
                            # chunk s's kT lands in SBUF via the Pool queue;
                            # absorb that dependency here, right before the
                            # first score matmul that needs it.
                            lo = s * (S // NCHUNK)
                            touch(kT_sb[:, 0, lo:lo + 2])
                        qt = qT_sb[:, :, s * QBLK:(s + 1) * QBLK]
                        po = [opsum.tile([128, DHALF], F32, name="po",
                                         tag="po") for _ in range(M)]
                        pd = dpsum.tile([128, 2 * M], F32, name="pd",
                                        tag="pd")

                        def consume(i, at, vts, hh, dst, fresh_touch):
                            # attn@v + denominator matmuls for visit i
                            vt = vts[i // 2]
                            if fresh_touch:
                                touch(vt[:, 0, 0:2])
                            for m in range(M):
                                nc.tensor.matmul(
                                    dst[m],
                                    at[:, m * 128:(m + 1) * 128],
                                    vt[:, i % 2, :],
                                    start=(i == 0), stop=(i == cov - 1))
                            if hh == 0:
                                for m in range(M):
                                    nc.tensor.matmul(
                                        pd[:, 2 * m:2 * m + 2],
                                        at[:, m * 128:(m + 1) * 128],
                                        ones_sb[:, :],
                                        start=(i == 0 and m == 0),
                                        stop=(i == cov - 1 and m == M - 1))

                        # ---- sweep 1: scores + exp + attn@v(d-half 0) ----
                        a_tiles = []
                        vts = {0: vt_load(0, 0)}
                        prev = None
                        for i in range(cov):
                            if i % 2 == 0 and i + 2 < cov:
                                vts[(i + 2) // 2] = vt_load(i + 2, 0)
                            ps = spsum.tile([128, QBLK], F32, name="ps",
                                            tag="ps")
                            for ci in range(DC):
                                nc.tensor.matmul(
                                    ps,
                                    kT_sb[:, ci, i * 128:(i + 1) * 128],
                                    qt[:, ci, :],
                                    start=(ci == 0), stop=(ci == DC - 1))
                            at = apool.tile([128, QBLK], BF16, name="at",
                                            tag="at")
                            nc.scalar.activation(
                                out=at, in_=ps, func=AF.Exp, scale=scale)
                            if i >= cov - 2 * M:
                                cm = cmpool.tile([128, QBLK], BF16,
                                                 name="cm", tag="cm")
                                nc.vector.tensor_scalar(
                                    out=cm, in0=ij_sb,
                                    scalar1=dl_sb[:, s * maxcov + i:
                                                  s * maxcov + i + 1],
                                    scalar2=None, op0=ALU.is_le)
                                nc.vector.tensor_mul(out=at, in0=at, in1=cm)
                            a_tiles.append(at)
                            if prev is not None:
                                consume(prev, a_tiles[prev], vts, 0, po,
                                        prev % 2 == 0)
                            prev = i
                        consume(prev, a_tiles[prev], vts, 0, po,
                                prev % 2 == 0)

                        rc = rcpool.tile([128, 2 * M], F32, name="rc",
                                         tag="rc")
                        nc.vector.reciprocal(out=rc, in_=pd)

                        # prefetch sweep-2's first v tiles BEFORE the output
                        # stores hit the same SP queue
                        vts2 = {0: vt_load(0, 1)}
                        if cov > 2:
                            vts2[1] = vt_load(2, 1)
                        if cov > 4:
                            vts2[2] = vt_load(4, 1)

                        def norm_store(dst_po, h):
                            # normalize on DVE (+ store via SP) so the ACT
                            # queue stays exp-only: the next slot's first exp
                            # must not sit behind norm/store work, or the PE
                            # stalls on the score-PSUM WAR.
                            for m in range(M):
                                ot = otpool.tile([128, DHALF], F32,
                                                 name="ot", tag="ot")
                                nc.vector.tensor_scalar(
                                    out=ot, in0=dst_po[m],
                                    scalar1=rc[:, 2 * m:2 * m + 1],
                                    scalar2=None, op0=ALU.mult)
                                nc.sync.dma_start(
                                    out=out_d[s * QBLK + m * 128:
                                              s * QBLK + (m + 1) * 128,
                                              h * DHALF:(h + 1) * DHALF],
                                    in_=ot)

                        norm_store(po, 0)

                        # ---- sweep 2: attn@v for remaining d-halves ----
                        for h in range(1, NH):
                            po2 = [opsum.tile([128, DHALF], F32, name="po",
                                              tag="po") for _ in range(M)]
                            for i in range(cov):
                                if i % 2 == 0 and i + 6 < cov:
                                    vts2[(i + 6) // 2] = vt_load(i + 6, h)
                                consume(i, a_tiles[i], vts2, h, po2,
                                        i % 2 == 0)
                            norm_store(po2, h)

            # collectives cannot live inside a hardware loop: unroll reps
            for rep in range(reps):
                body(rep)
    nc.compile()
    return nc


def host_core_inputs(x_b, Wq, Wk, Wv, q0s, S, D, QBLK, COV, half=None):
    """Input map for one core.  half=0/1 selects which half of each exchange
    chunk this core projects (None: full-sequence kv projection)."""
    nslots = len(COV)
    maxcov = max(COV)
    if half is None:
        xkv = x_b
    else:
        CROWS = S // NCHUNK
        HROWS = CROWS // 2
        rows = []
        for c in range(NCHUNK):
            lo = c * CROWS + half * HROWS
            rows.append(x_b[lo:lo + HROWS])
        xkv = np.concatenate(rows, axis=0)
    xTkv = np.ascontiguousarray(xkv.T.astype(bfloat16))
    xTq = np.ascontiguousarray(
        np.concatenate([x_b[q0:q0 + QBLK] for q0 in q0s], axis=0).T
        .astype(bfloat16))
    ij = (np.arange(128, dtype=np.float32)[:, None]
          - np.arange(QBLK, dtype=np.float32)[None, :])
    ij = np.ascontiguousarray(np.broadcast_to(ij, (128, QBLK)))
    delta = np.empty((nslots, maxcov), dtype=np.float32)
    for s, q0 in enumerate(q0s):
        delta[s, :] = q0 - 128.0 * np.arange(maxcov, dtype=np.float32)
    delta = np.ascontiguousarray(
        np.broadcast_to(delta.reshape(1, -1), (128, nslots * maxcov)))
    return {
        "xTkv": xTkv, "xTq": xTq,
        "Wq": np.ascontiguousarray(Wq.astype(bfloat16)),
        "Wk": np.ascontiguousarray(Wk.astype(bfloat16)),
        "Wv": np.ascontiguousarray(Wv.astype(bfloat16)),
        "IJ": ij, "delta": delta,
        "ones": np.ones((128, 2), bfloat16),
    }


_NC_CACHE = {}


def _get_nc(key, cfg, **kw):
    if key not in _NC_CACHE:
        _NC_CACHE[key] = build_nc(**cfg, **kw)
    return _NC_CACHE[key]


def run_full(x, Wq, Wk, Wv, trace=False, trace_cores=None):
    """Run the 8-core kernel on full inputs; returns (out, BassKernelResults)."""
    cfg = CFG_FULL
    S, D, QBLK, COV = cfg["S"], cfg["D"], cfg["QBLK"], cfg["COV"]
    x = np.asarray(x, np.float32)
    Wq = np.asarray(Wq, np.float32)
    Wk = np.asarray(Wk, np.float32)
    Wv = np.asarray(Wv, np.float32)
    B = x.shape[0]
    assert (B, x.shape[1], x.shape[2]) == (B_FULL, S, D)

    rg = RG_FULL if USE_RG else None
    nc = _get_nc("full", cfg, rg=rg)
    in_maps = []
    for b in range(B):
        for h in range(2):
            in_maps.append(host_core_inputs(
                x[b], Wq, Wk, Wv, Q0_FULL[h], S, D, QBLK, COV,
                half=(h if rg else None)))
    res = run_bass_kernel_spmd(
        nc, in_maps, list(range(2 * B)), trace=trace,
        trace_cores=trace_cores)
    out = np.empty((B, S, D), np.float32)
    for b in range(B):
        for h in range(2):
            o = np.asarray(res.results[2 * b + h]["out"])
            for s, q0 in enumerate(Q0_FULL[h]):
                out[b, q0:q0 + QBLK] = o[s * QBLK:(s + 1) * QBLK]
    return out, res


def kernel(x, Wq, Wk, Wv):
    out, _ = run_full(x, Wq, Wk, Wv)
    return out
